# revision 2
# baseline (speedup 1.0000x reference)
"""Bass/Trainium2 kernel for 7x7 valid cross-correlation on a 8192x8192 fp32 image.

Sharding: output columns split across 8 NeuronCores (1024 cols/core plus a
6-col halo); kernel and bias replicated. All image/band/output traffic is
bf16 (host casts both ways), halving DMA to ~34MB/core so the TensorEngine
is the only roofline.

Main loop (rows 0..8173): 67 groups x 122 output rows, each as 2x512-free
column tiles of 7 PSUM-accumulated banded-Toeplitz matmuls (stationary
B[k, j*128+m] = w[k-m, j]; moving = x shifted by column tap j), 213ns each
at 1 row/cycle.

Remainder strip (rows 8174..8185): computed transposed so its matmul free
dim is just 12 rows: a 32-row x-slice is PE-transposed (identity permutation)
into column-major tiles, then a row-tap band C[k, d*128+c] = w[d, k-c]
accumulates 7 taps per 128-column window - 63 matmuls x 5ns instead of a full
14 x 512-free group (~3us saved). Results land in ystrip[1024, 12], which the
host transposes into the output.

Scheduling: image loads ride the Pool-engine SWDGE (no HWDGE contention);
stores ride the sync ring; B/C/bias/strip-source ride sync+scalar. Startup
loads B taps 0-1 + x cols 0:774 first and runs group 0 as four 256-free
chunks interleaved by tap, so the PE starts at ~3.9us and never stalls while
the rest streams in. The strip is interleaved with groups 13-22 to hide its
PSUM->SBUF latency, and the last group ends with a 128-free chunk whose
258ns bias and small final store are the only post-PE work.
"""

import numpy as np
import ml_dtypes

import concourse.bacc as bacc
import concourse.tile as tile
import concourse.mybir as mybir
from concourse.bass_utils import run_bass_kernel_spmd

H = W = 8192
KH = KW = 7
OH = OW = H - KH + 1  # 8186

N_CORES = 8
COLS_PER_CORE = 1024
IN_COLS = COLS_PER_CORE + KW - 1  # 1030

GROUP = 122
NTILE = 512
N_FULL_ROWS = 67 * GROUP        # 8174 rows handled row-major
STRIP_ROWS = OH - N_FULL_ROWS   # 12 remainder rows, handled transposed
GROUP_STARTS = list(range(0, N_FULL_ROWS, GROUP))

# Transposed-strip column windows: starts of the 128-wide x-column windows
# (partition dim after DMA transpose), the output-column start each window
# serves, and the width served. delta = m0 - x0 is the column offset into the
# C band.
STRIP_WINS = [(122 * t, 122 * t, 122) for t in range(8)] + [(902, 976, 48)]
STRIP_SRC_ROW0 = H - 32         # 8160: 32-row window covering rows 8174..8191

MM_DT = mybir.dt.bfloat16
F32 = mybir.dt.float32


def _build_nc():
    nc = bacc.Bacc(
        "TRN2", target_bir_lowering=False, debug=False, num_devices=N_CORES
    )
    x = nc.dram_tensor("x", [H, IN_COLS], MM_DT, kind="ExternalInput").ap()
    B = nc.dram_tensor("B", [128, KW * 128], MM_DT, kind="ExternalInput").ap()
    C = nc.dram_tensor("C", [128, KH * 128], MM_DT, kind="ExternalInput").ap()
    ident = nc.dram_tensor("ident", [32, 32], MM_DT, kind="ExternalInput").ap()
    bias = nc.dram_tensor("bias", [128, 1], F32, kind="ExternalInput").ap()
    y = nc.dram_tensor("y", [N_FULL_ROWS, COLS_PER_CORE], MM_DT, kind="ExternalOutput").ap()
    ystrip = nc.dram_tensor(
        "ystrip", [COLS_PER_CORE, STRIP_ROWS], MM_DT, kind="ExternalOutput"
    ).ap()

    with tile.TileContext(nc) as tc:
        with (
            tc.tile_pool(name="sb", bufs=1) as sb,
            tc.tile_pool(name="psum", bufs=8, space="PSUM") as psum_pool,
        ):
            B_sb = sb.tile([128, KW * 128], MM_DT, name="B_sb", tag="c", bufs=1)
            C_sb = sb.tile([128, KH * 128], MM_DT, name="C_sb", tag="cc", bufs=1)
            bias_sb = sb.tile([128, 1], F32, name="bias_sb", tag="cb", bufs=1)
            x0_sb = sb.tile([128, IN_COLS], MM_DT, name="x", tag="x", bufs=4)
            xt_sb = [
                sb.tile([128, 32], MM_DT, name=f"xt{t}", tag="xt", bufs=9)
                for t in range(9)
            ]
            xs_sb = sb.tile([32, IN_COLS], MM_DT, name="xs", tag="xs", bufs=1)
            id_sb = sb.tile([32, 32], MM_DT, name="id_sb", tag="id", bufs=1)

            # Startup-critical loads: B taps 0-1 (sync), x cols 0:774 (Pool),
            # B taps 2-6 (sync), rest of x (Pool). Scalar ring carries bias,
            # C, the identity, and the 32-row strip source - needed later.
            nc.sync.dma_start(B_sb[:, 0:256], B[:, 0:256])
            nc.gpsimd.dma_start(x0_sb[:, 0:774], x[0:128, 0:774])
            nc.sync.dma_start(B_sb[:, 256:], B[:, 256:])
            nc.gpsimd.dma_start(x0_sb[:, 774:], x[0:128, 774:])
            nc.scalar.dma_start(bias_sb[:], bias[:])
            nc.scalar.dma_start(C_sb[:], C[:])
            nc.scalar.dma_start(id_sb[:], ident[:])
            nc.scalar.dma_start(
                xs_sb[:, :], x[STRIP_SRC_ROW0 : STRIP_SRC_ROW0 + 32, :]
            )

            def emit_transpose(t):
                # PE transpose: xt[c, r] = xs[r, x0+c] via identity permutation
                x0 = STRIP_WINS[t][0]
                pt = psum_pool.tile([128, 32], MM_DT, name=f"ptr{t}", tag="ps")
                nc.tensor.matmul(
                    pt[:, :],
                    xs_sb[0:32, x0 : x0 + 128],
                    id_sb[0:32, 0:32],
                    start=True,
                    stop=True,
                    is_transpose=True,
                )
                nc.vector.tensor_copy(xt_sb[t][:, :], pt[:, :])

            def mm(ps, x_sb, krows, mcols, c0, fsz, j):
                nc.tensor.matmul(
                    ps[0:mcols, 0:fsz],
                    B_sb[0:krows, j * 128 : j * 128 + mcols],
                    x_sb[0:krows, c0 + j : c0 + j + fsz],
                    start=(j == 0),
                    stop=(j == KW - 1),
                )

            def bias_to(o_sb, ps, grows, c0, fsz):
                nc.vector.tensor_scalar_add(
                    o_sb[0:grows, c0 : c0 + fsz], ps[0:grows, 0:fsz],
                    bias_sb[0:grows, 0:1]
                )

            def emit_strip_group(t):
                # Transposed remainder strip, one column window: 7 row-tap
                # matmuls with free dim = the 12 output rows.
                # xt[kappa, rr] = x[8160+rr, x0+kappa]; output rows r use
                # rr = r + d + (8174-8160).
                roff = N_FULL_ROWS - STRIP_SRC_ROW0  # 8174-8160 = 14
                x0, m0, mwid = STRIP_WINS[t]
                delta = m0 - x0
                pst = psum_pool.tile(
                    [128, STRIP_ROWS], F32, name=f"pst{t}", tag="ps"
                )
                for d in range(KH):
                    nc.tensor.matmul(
                        pst[0:mwid, :],
                        C_sb[:, d * 128 + delta : d * 128 + delta + mwid],
                        xt_sb[t][:, roff + d : roff + d + STRIP_ROWS],
                        start=(d == 0),
                        stop=(d == KH - 1),
                    )
                ot = sb.tile(
                    [128, STRIP_ROWS], MM_DT, name=f"ot{t}", tag="ot", bufs=9
                )
                nc.vector.tensor_scalar_add(
                    ot[0:mwid, :], pst[0:mwid, :], bias_sb[0:mwid, 0:1]
                )
                nc.scalar.dma_start(ystrip[m0 : m0 + mwid, :], ot[0:mwid, :])

            for gi, g0 in enumerate(GROUP_STARTS):
                grows = GROUP
                krows = grows + KH - 1
                mcols = 128
                lastg = gi == len(GROUP_STARTS) - 1

                if gi == 0:
                    x_sb = x0_sb
                else:
                    x_sb = sb.tile([128, IN_COLS], MM_DT, name="x", tag="x", bufs=4)
                    nc.gpsimd.dma_start(x_sb[0:krows, :], x[g0 : g0 + krows, :])
                o_sb = sb.tile([128, COLS_PER_CORE], MM_DT, name="o", tag="o", bufs=8)

                if gi == 0:
                    chunks = [
                        psum_pool.tile([128, 256], F32, name=f"p{i}", tag="ps")
                        for i in range(4)
                    ]
                    for j in range(KW):
                        for ci in range(3):
                            mm(chunks[ci], x_sb, krows, mcols, 256 * ci, 256, j)
                    for j in range(KW):
                        mm(chunks[3], x_sb, krows, mcols, 768, 256, j)
                    for ci in range(4):
                        bias_to(o_sb, chunks[ci], grows, 256 * ci, 256)
                    nc.sync.dma_start(y[g0 : g0 + grows, :], o_sb[0:grows, :])
                elif not lastg:
                    for c0 in (0, 512):
                        ps = psum_pool.tile([128, NTILE], F32, name="ps", tag="ps")
                        for j in range(KW):
                            mm(ps, x_sb, krows, mcols, c0, NTILE, j)
                        bias_to(o_sb, ps, grows, c0, NTILE)
                    nc.sync.dma_start(y[g0 : g0 + grows, :], o_sb[0:grows, :])
                else:
                    # final row-major group: shrinking chunks + two stores so
                    # the post-PE chain is one 258ns bias + one tiny store
                    for c0, fsz in ((0, 512), (512, 256), (768, 128), (896, 128)):
                        ps = psum_pool.tile([128, fsz], F32, name="pt", tag="ps")
                        for j in range(KW):
                            mm(ps, x_sb, krows, mcols, c0, fsz, j)
                        bias_to(o_sb, ps, grows, c0, fsz)
                        if c0 == 0:
                            nc.sync.dma_start(
                                y[g0 : g0 + grows, 0:512], o_sb[0:grows, 0:512]
                            )
                        elif c0 == 512:
                            nc.sync.dma_start(
                                y[g0 : g0 + grows, 512:768], o_sb[0:grows, 512:768]
                            )
                    nc.sync.dma_start(
                        y[g0 : g0 + grows, 768:1024], o_sb[0:grows, 768:1024]
                    )

                if 13 <= gi <= 21:
                    # one strip window per main group: the PE transpose's
                    # PSUM->DVE->SBUF round-trip hides under the next group's
                    # ~3us of matmuls, then the 7 tiny strip matmuls run
                    emit_transpose(gi - 13)
                if 14 <= gi <= 22:
                    emit_strip_group(gi - 14)

    nc.compile()
    return nc


_NC_CACHE = None


def _get_nc():
    global _NC_CACHE
    if _NC_CACHE is None:
        _NC_CACHE = _build_nc()
    return _NC_CACHE


def make_in_maps(x, weight, bias):
    x = np.asarray(x, dtype=np.float32)
    weight = np.asarray(weight, dtype=np.float32)
    bias = np.asarray(bias, dtype=np.float32)

    # Column-tap bands (stationary for row-major groups):
    # B[k, j*128 + m] = weight[k-m, j], 0 <= k-m < KH.
    B = np.zeros((128, KW * 128), dtype=np.float32)
    m = np.arange(GROUP)
    for j in range(KW):
        for d in range(KH):
            B[m + d, j * 128 + m] = weight[d, j]
    B = B.astype(ml_dtypes.bfloat16)

    # Row-tap bands (stationary for the transposed strip):
    # C[k, d*128 + c] = weight[d, k-c], 0 <= k-c < KW, over the full 128 cols.
    C = np.zeros((128, KH * 128), dtype=np.float32)
    c = np.arange(128)
    for d in range(KH):
        for j in range(KW):
            kk = c + j
            sel = kk < 128
            C[kk[sel], d * 128 + c[sel]] = weight[d, j]
    C = C.astype(ml_dtypes.bfloat16)

    bias_bcast = np.full((128, 1), bias[0], dtype=np.float32)
    ident = np.eye(32, dtype=ml_dtypes.bfloat16)

    x_bf = x.astype(ml_dtypes.bfloat16)
    x_pad = np.concatenate(
        [x_bf, np.zeros((H, KW - 1), dtype=ml_dtypes.bfloat16)], axis=1
    )
    return [
        {
            "x": np.ascontiguousarray(
                x_pad[:, cc * COLS_PER_CORE : cc * COLS_PER_CORE + IN_COLS]
            ),
            "B": B,
            "C": C,
            "ident": ident,
            "bias": bias_bcast,
        }
        for cc in range(N_CORES)
    ]


def kernel(x: np.ndarray, weight: np.ndarray, bias: np.ndarray) -> np.ndarray:
    in_maps = make_in_maps(x, weight, bias)
    nc = _get_nc()
    res = run_bass_kernel_spmd(nc, in_maps, core_ids=list(range(N_CORES)))
    full = np.empty((OH, N_CORES * COLS_PER_CORE), dtype=np.float32)
    for cc in range(N_CORES):
        sl = slice(cc * COLS_PER_CORE, (cc + 1) * COLS_PER_CORE)
        full[:N_FULL_ROWS, sl] = np.asarray(res.results[cc]["y"]).astype(np.float32)
        full[N_FULL_ROWS:, sl] = (
            np.asarray(res.results[cc]["ystrip"]).astype(np.float32).T
        )
    return np.ascontiguousarray(full[:, :OW])
